# revision 1
# baseline (speedup 1.0000x reference)
"""BiMamba (fwd+bwd Mamba + merge) Trainium2 Bass kernel.

Sharding (8 cores): core = batch*4 + dir*2 + e_half.
Each core computes one (batch, direction) pair over 1024 of the 2048 d_inner
channels, in e-partition layout [e_p=128 x 8 tiles, t_free=1024].
bwd cores operate entirely in flipped time (host pre-flips x); the final
out_proj partial is un-flipped via a data-driven mask combine, then a 4-core
AllReduce produces the full (d, t) output on every core of the batch group.

Self-contained: hardcodes B=2, L=1024, D=1024, E=2048 (1024/core), N=16,
dt_rank=64, d_conv=4.
"""
import numpy as np

B, L, D = 2, 1024, 1024
E = 2048
EH = 1024            # channels per core (half of E)
N = 16
DTR = 64
K = 4                # d_conv
M_TILES = 8          # e-tiles per core
NB = 8               # n-plane batches
NPB = 2              # planes per batch
PL = L + 2           # plane stride with 2-col zero gap for the batched scan

_nc_cache = {}


def _build_nc():
    import concourse.bacc as bacc
    import concourse.mybir as mybir
    from concourse import tile

    f32, f16 = mybir.dt.float32, mybir.dt.float16
    Alu = mybir.AluOpType
    Act = mybir.ActivationFunctionType

    nc = bacc.Bacc("TRN2", target_bir_lowering=False, debug=False, num_devices=8)

    # ---- DRAM I/O ----
    xT_d = nc.dram_tensor("xT", [D, 3 + L], f16, kind="ExternalInput")
    # pre-tiled: [p, m*1024 + kt*128 + e']  (one DMA per m-slab)
    wxiT_d = nc.dram_tensor("wxiT", [128, M_TILES * EH], f16, kind="ExternalInput")
    wzT_d = nc.dram_tensor("wzT", [128, M_TILES * EH], f16, kind="ExternalInput")
    convw_d = nc.dram_tensor("convw", [128, M_TILES * K], f32, kind="ExternalInput")
    convb_d = nc.dram_tensor("convb", [128, M_TILES], f32, kind="ExternalInput")
    xpT_d = nc.dram_tensor("xpT", [EH, 96], f16, kind="ExternalInput")
    dtwT_d = nc.dram_tensor("dtwT", [DTR, EH], f32, kind="ExternalInput")
    dtb_d = nc.dram_tensor("dtb", [128, M_TILES], f32, kind="ExternalInput")
    arate_d = nc.dram_tensor("arate", [128, M_TILES * N], f32, kind="ExternalInput")
    dp_d = nc.dram_tensor("dp", [128, M_TILES], f32, kind="ExternalInput")
    # pre-tiled: [p, dm*1024 + m*128 + d']
    woT_d = nc.dram_tensor("woT", [128, M_TILES * D], f16, kind="ExternalInput")
    mf_d = nc.dram_tensor("mf", [128, 1], f32, kind="ExternalInput")
    mb_d = nc.dram_tensor("mb", [128, 1], f32, kind="ExternalInput")

    dbl_in = nc.dram_tensor("dbl_in", [64, L], f32, kind="Internal")
    dbl_out = nc.dram_tensor("dbl_out", [64, L], f32, kind="Internal")
    bc16_in = nc.dram_tensor("bc16_in", [32, L], f16, kind="Internal")
    bc16_d = nc.dram_tensor("bc16", [32, L], f16, kind="Internal")
    oc_in = nc.dram_tensor("oc_in", [D, L], f16, kind="Internal")
    oc_out = nc.dram_tensor("oc_out", [256, L], f16, kind="Internal")
    out_d = nc.dram_tensor("out_p", [256, L], f16, kind="ExternalOutput")

    with tile.TileContext(nc) as tc:
        with tc.tile_pool(name="const", bufs=1) as cpool, \
             tc.tile_pool(name="res", bufs=1) as rpool:
            convw = cpool.tile([128, M_TILES * K], f32)
            convb = cpool.tile([128, M_TILES], f32)
            dtb = cpool.tile([128, M_TILES], f32)
            arate = cpool.tile([128, M_TILES * N], f32)
            dp = cpool.tile([128, M_TILES], f32)
            mf = cpool.tile([128, 1], f32)
            mb = cpool.tile([128, 1], f32)
            for t_, d_ in ((convw, convw_d), (convb, convb_d), (dtb, dtb_d),
                           (arate, arate_d), (dp, dp_d), (mf, mf_d), (mb, mb_d)):
                nc.sync.dma_start(t_[:], d_[:])

            xc16 = rpool.tile([128, M_TILES * L], f16)
            sz16 = rpool.tile([128, M_TILES * L], f16)
            g16 = rpool.tile([128, M_TILES * L], f16)
            bca = rpool.tile([128, N * L], f16)
            bcc = rpool.tile([128, N * L], f16)
            dblr = rpool.tile([64, L], f32)

            # ---------- Phase A: in_proj matmuls + conv + silu ----------
            with tc.tile_pool(name="pa", bufs=1) as pap, \
                 tc.tile_pool(name="paw", bufs=4) as pwp, \
                 tc.tile_pool(name="pax", bufs=2) as pxp, \
                 tc.tile_pool(name="psA", bufs=2, space="PSUM") as psA:
                xT = pap.tile([128, M_TILES * (3 + L)], f16)
                dma_engs = [nc.sync, nc.scalar, nc.gpsimd]
                for kt in range(M_TILES):
                    dma_engs[kt % 3].dma_start(xT[:, kt * (3 + L):(kt + 1) * (3 + L)],
                                               xT_d[kt * 128:(kt + 1) * 128, :])
                for m in range(M_TILES):
                    wxi = pwp.tile([128, M_TILES * 128], f16, tag="wxi")
                    wz = pwp.tile([128, M_TILES * 128], f16, tag="wz")
                    nc.scalar.dma_start(wxi[:], wxiT_d[:, m * EH:(m + 1) * EH])
                    nc.gpsimd.dma_start(wz[:], wzT_d[:, m * EH:(m + 1) * EH])
                    ps_xi = psA.tile([128, L], f32, tag="xi")
                    ps_z = psA.tile([128, L], f32, tag="z")
                    for kt in range(M_TILES):
                        xk = xT[:, kt * (3 + L):(kt + 1) * (3 + L)]
                        for h in range(2):
                            nc.tensor.matmul(ps_xi[:, h * 512:(h + 1) * 512],
                                             wxi[:, kt * 128:(kt + 1) * 128],
                                             xk[:, 3 + h * 512: 3 + (h + 1) * 512],
                                             start=(kt == 0), stop=(kt == M_TILES - 1))
                            nc.tensor.matmul(ps_z[:, h * 512:(h + 1) * 512],
                                             wz[:, kt * 128:(kt + 1) * 128],
                                             xk[:, 3 + h * 512: 3 + (h + 1) * 512],
                                             start=(kt == 0), stop=(kt == M_TILES - 1))
                    # conv: xi32 padded copy, then 4-tap chain on DVE
                    xi32 = pxp.tile([128, 3 + L], f32, tag="xi32")
                    nc.vector.memset(xi32[:, 0:3], 0.0)
                    nc.scalar.copy(xi32[:, 3:3 + L], ps_xi[:])
                    cacc = pxp.tile([128, L], f32, tag="cacc")
                    nc.vector.tensor_scalar_mul(cacc[:], xi32[:, 0:L], convw[:, m * K:m * K + 1])
                    for k in range(1, K):
                        nc.vector.scalar_tensor_tensor(
                            cacc[:], xi32[:, k:k + L], convw[:, m * K + k:m * K + k + 1],
                            cacc[:], Alu.mult, Alu.add)
                    nc.scalar.activation(xc16[:, m * L:(m + 1) * L], cacc[:],
                                         Act.Silu, bias=convb[:, m:m + 1])
                    nc.scalar.activation(sz16[:, m * L:(m + 1) * L], ps_z[:], Act.Silu)

            # ---------- Phase B: x_proj partial + AllReduce + broadcasts ----------
            with tc.tile_pool(name="pb", bufs=1) as pbp, \
                 tc.tile_pool(name="pbw", bufs=2) as pbw, \
                 tc.tile_pool(name="psB", bufs=1, space="PSUM") as psB:
                ps_dbl = psB.tile([96, L], f32)
                for m in range(M_TILES):
                    xp = pbw.tile([128, 96], f16, tag="xp")
                    nc.sync.dma_start(xp[:], xpT_d[m * 128:(m + 1) * 128, :])
                    for h in range(2):
                        nc.tensor.matmul(ps_dbl[:, h * 512:(h + 1) * 512], xp[:],
                                         xc16[:, m * L + h * 512: m * L + (h + 1) * 512],
                                         start=(m == 0), stop=(m == M_TILES - 1))
                # split: dt rows AllReduce in f32; B/C rows in f16 (feeds planes)
                dbl_sb = pbp.tile([64, L], f32)
                nc.vector.tensor_copy(dbl_sb[:], ps_dbl[0:64, :])
                nc.sync.dma_start(dbl_in[:], dbl_sb[:])
                cvt16 = pbp.tile([128, L], f16)
                nc.vector.tensor_copy(cvt16[64:96, :], ps_dbl[64:96, :])
                nc.scalar.dma_start(bc16_in[:], cvt16[64:96, :])
                nc.gpsimd.collective_compute(
                    "AllReduce", Alu.add,
                    replica_groups=[[0, 1], [2, 3], [4, 5], [6, 7]],
                    ins=[bc16_in[:]], outs=[bc16_d[:]])
                nc.gpsimd.collective_compute(
                    "AllReduce", Alu.add,
                    replica_groups=[[0, 1], [2, 3], [4, 5], [6, 7]],
                    ins=[dbl_in[:]], outs=[dbl_out[:]])
                nc.sync.dma_start(dblr[0:64, :], dbl_out[:])
                engs = [nc.sync, nc.scalar, nc.gpsimd]
                for n in range(N):
                    engs[n % 3].dma_start(bca[:, n * L:(n + 1) * L],
                                          bc16_d[n:n + 1, :].broadcast_to([128, L]))
                    engs[(n + 1) % 3].dma_start(bcc[:, n * L:(n + 1) * L],
                                                bc16_d[N + n:N + n + 1, :].broadcast_to([128, L]))

            # ---------- Phase C: delta, dA planes, scan, y ----------
            with tc.tile_pool(name="pc", bufs=2) as pcp, \
                 tc.tile_pool(name="pc1", bufs=1) as pc1, \
                 tc.tile_pool(name="psC", bufs=2, space="PSUM") as psC:
                bca3 = bca[:].rearrange("p (n l) -> p n l", l=L)
                bcc3 = bcc[:].rearrange("p (n l) -> p n l", l=L)
                for m in range(M_TILES):
                    dtw = pcp.tile([DTR, 128], f32, tag="dtw")
                    nc.sync.dma_start(dtw[:], dtwT_d[:, m * 128:(m + 1) * 128])
                    ps_dt = psC.tile([128, L], f32, tag="dt")
                    for h in range(2):
                        nc.tensor.matmul(ps_dt[:, h * 512:(h + 1) * 512], dtw[:],
                                         dblr[0:DTR, h * 512:(h + 1) * 512],
                                         start=True, stop=True)
                    # softplus(raw) = ln(1 + exp(raw)); Softplus has no act table here
                    delta32 = pcp.tile([128, L], f32, tag="d32")
                    delta16 = pcp.tile([128, L], f16, tag="d16")
                    ee = pcp.tile([128, L], f32, tag="ee")
                    nc.scalar.activation(ee[:], ps_dt[:], Act.Exp, bias=dtb[:, m:m + 1])
                    nc.scalar.activation(delta32[:], ee[:], Act.Ln, bias=1.0)
                    nc.vector.tensor_copy(delta16[:], delta32[:])
                    u16 = pcp.tile([128, L], f16, tag="u16")
                    nc.vector.tensor_mul(u16[:], delta16[:], xc16[:, m * L:(m + 1) * L])
                    yparts = pc1.tile([128, NB * L], f16, tag="yp")
                    for nb in range(NB):
                        dA = pcp.tile([128, NPB * PL], f32, tag="dA")
                        dBu = pcp.tile([128, NPB * PL], f16, tag="dBu")
                        for j in range(NPB):
                            n = nb * NPB + j
                            nc.scalar.activation(dA[:, j * PL:j * PL + L], delta32[:],
                                                 Act.Exp, scale=arate[:, m * N + n:m * N + n + 1])
                        dA3 = dA[:].rearrange("p (n l) -> p n l", l=PL)
                        dBu3 = dBu[:].rearrange("p (n l) -> p n l", l=PL)
                        if m == 0 and nb < 2:
                            # gap columns stay 0 across slot reuse (2 slots/tag)
                            nc.vector.memset(dA3[:, :, L:PL], 0.0)
                            nc.vector.memset(dBu3[:, :, L:PL], 0.0)
                        nc.vector.tensor_mul(
                            dBu3[:, :, 0:L],
                            u16[:, None, :].broadcast_to([128, NPB, L]),
                            bca3[:, nb * NPB:(nb + 1) * NPB, :])
                        h4 = pcp.tile([128, NPB * PL], f16, tag="h4")
                        nc.vector.tensor_tensor_scan(h4[:], dA[:], dBu[:], 0.0,
                                                     Alu.mult, Alu.add)
                        h43 = h4[:].rearrange("p (n l) -> p n l", l=PL)
                        prod = pcp.tile([128, NPB * PL], f16, tag="dBu")
                        prod3 = prod[:].rearrange("p (n l) -> p n l", l=PL)
                        nc.vector.tensor_mul(prod3[:, :, 0:L], h43[:, :, 0:L],
                                             bcc3[:, nb * NPB:(nb + 1) * NPB, :])
                        nc.vector.tensor_add(yparts[:, nb * L:(nb + 1) * L],
                                             prod[:, 0:L], prod[:, PL:PL + L])
                    t4 = pc1.tile([128, 4 * L], f16, tag="t4")
                    nc.vector.tensor_add(t4[:], yparts[:, 0:4 * L], yparts[:, 4 * L:8 * L])
                    t2 = pc1.tile([128, 2 * L], f16, tag="t2")
                    nc.vector.tensor_add(t2[:], t4[:, 0:2 * L], t4[:, 2 * L:4 * L])
                    y16 = pc1.tile([128, L], f16, tag="y16")
                    nc.vector.tensor_add(y16[:], t2[:, 0:L], t2[:, L:2 * L])
                    ys16 = pc1.tile([128, L], f16, tag="ys16")
                    nc.vector.scalar_tensor_tensor(ys16[:], xc16[:, m * L:(m + 1) * L],
                                                   dp[:, m:m + 1], y16[:], Alu.mult, Alu.add)
                    nc.vector.tensor_mul(g16[:, m * L:(m + 1) * L], ys16[:],
                                         sz16[:, m * L:(m + 1) * L])

            # ---------- Phase D: out_proj + flip-combine + AllReduce ----------
            with tc.tile_pool(name="pd", bufs=2) as pdp, \
                 tc.tile_pool(name="psD", bufs=2, space="PSUM") as psD:
                for dm in range(M_TILES):
                    wo = pdp.tile([128, M_TILES * 128], f16, tag="wo")
                    nc.sync.dma_start(wo[:], woT_d[:, dm * D:(dm + 1) * D])
                    ps_o = psD.tile([128, L], f32, tag="o")
                    for m in range(M_TILES):
                        for h in range(2):
                            nc.tensor.matmul(ps_o[:, h * 512:(h + 1) * 512],
                                             wo[:, m * 128:(m + 1) * 128],
                                             g16[:, m * L + h * 512: m * L + (h + 1) * 512],
                                             start=(m == 0), stop=(m == M_TILES - 1))
                    o32 = pdp.tile([128, L], f32, tag="o32")
                    nc.vector.tensor_copy(o32[:], ps_o[:])
                    t1 = pdp.tile([128, L], f16, tag="t1")
                    nc.vector.tensor_scalar_mul(t1[:], o32[:], mf[:, 0:1])
                    ocs = pdp.tile([128, L], f16, tag="ocs")
                    nc.vector.scalar_tensor_tensor(ocs[:], o32[:, ::-1], mb[:, 0:1],
                                                   t1[:], Alu.mult, Alu.add)
                    nc.sync.dma_start(oc_in[dm * 128:(dm + 1) * 128, :], ocs[:])
                    if dm % 2 == 1:
                        # ReduceScatter: group-rank ci gets a contiguous 64-row
                        # shard of each 256-row chunk; host stitches shards.
                        ch = dm // 2
                        nc.gpsimd.collective_compute(
                            "ReduceScatter", Alu.add,
                            replica_groups=[[0, 1, 2, 3], [4, 5, 6, 7]],
                            ins=[oc_in[ch * 256:(ch + 1) * 256, :]],
                            outs=[oc_out[ch * 64:(ch + 1) * 64, :]])
                        nc.sync.dma_start(out_d[ch * 64:(ch + 1) * 64, :],
                                          oc_out[ch * 64:(ch + 1) * 64, :])

    nc.compile()
    return nc


def _host_prep(inputs):
    """Build the 8 per-core input maps from the full problem inputs."""
    x = np.asarray(inputs["x"], np.float32)
    merge_w = np.asarray(inputs["merge_w"], np.float32)
    in_maps = []
    for b in range(B):
        for di, pre in enumerate(("fwd", "bwd")):
            p = {k: np.asarray(inputs[f"{pre}_{k}"], np.float32)
                 for k in ("in_proj", "conv_w", "conv_b", "x_proj", "dt_w",
                           "dt_b", "A_log", "D", "out_proj")}
            xb = x[b]
            if di == 1:
                xb = xb[::-1]
            xTp = np.concatenate([np.zeros((D, 3), np.float32), xb.T], axis=1)
            A = -np.exp(p["A_log"])                       # (E, N)
            W = merge_w[:, di * D:(di + 1) * D] @ p["out_proj"]   # (D, E)
            def pack_lhsT(wT):
                # (D, EH) -> [p, m*1024 + kt*128 + e']
                return np.ascontiguousarray(
                    wT.reshape(M_TILES, 128, M_TILES, 128).transpose(1, 2, 0, 3)
                    .reshape(128, M_TILES * EH))

            for half in range(2):
                sl = slice(half * EH, (half + 1) * EH)
                wxiT = pack_lhsT(p["in_proj"][:E][sl].T)
                wzT = pack_lhsT(p["in_proj"][E:][sl].T)
                convw = p["conv_w"][sl].reshape(M_TILES, 128, K).transpose(1, 0, 2).reshape(128, M_TILES * K)
                convb = p["conv_b"][sl].reshape(M_TILES, 128).T
                xpT = p["x_proj"][:, sl].T                # (EH, 96)
                dtwT = p["dt_w"][sl].T                    # (DTR, EH)
                dtb = p["dt_b"][sl].reshape(M_TILES, 128).T
                arate = A[sl].reshape(M_TILES, 128, N).transpose(1, 0, 2).reshape(128, M_TILES * N)
                dpv = p["D"][sl].reshape(M_TILES, 128).T
                woT = pack_lhsT(W[:, sl].T)               # (EH, D) pre-tiled
                fwd = (di == 0)
                in_maps.append({
                    "xT": xTp.astype(np.float16),
                    "wxiT": wxiT.astype(np.float16),
                    "wzT": wzT.astype(np.float16),
                    "convw": np.ascontiguousarray(convw, np.float32),
                    "convb": np.ascontiguousarray(convb, np.float32),
                    "xpT": xpT.astype(np.float16),
                    "dtwT": np.ascontiguousarray(dtwT, np.float32),
                    "dtb": np.ascontiguousarray(dtb, np.float32),
                    "arate": np.ascontiguousarray(arate, np.float32),
                    "dp": np.ascontiguousarray(dpv, np.float32),
                    "woT": woT.astype(np.float16),
                    "mf": np.full((128, 1), 1.0 if fwd else 0.0, np.float32),
                    "mb": np.full((128, 1), 0.0 if fwd else 1.0, np.float32),
                })
    return in_maps


def _ensure_neuron_platform():
    """If a caller pinned jax to cpu, re-point it at the neuron/axon PJRT
    platform so run_bass_kernel_spmd sees the 8 NeuronCores."""
    import jax
    try:
        if len(jax.devices()) >= 8 and jax.devices()[0].platform != "cpu":
            return
    except Exception:
        pass
    for plat in ("axon", "neuron"):
        try:
            jax.config.update("jax_platforms", plat)
            if len(jax.devices()) >= 8:
                return
        except Exception:
            continue


def kernel(**inputs):
    _ensure_neuron_platform()
    from concourse.bass_utils import run_bass_kernel_spmd
    if "nc" not in _nc_cache:
        _nc_cache["nc"] = _build_nc()
    nc = _nc_cache["nc"]
    in_maps = _host_prep(inputs)
    res = run_bass_kernel_spmd(nc, in_maps, core_ids=list(range(8)))
    _nc_cache["last_results"] = res
    # Stitch ReduceScatter shards: 4 chunks of 256 d-rows; within chunk ch,
    # group-rank ci holds rows [ch*256 + ci*64 : +64] at out_p[ch*64:(ch+1)*64].
    out = np.zeros((B, L, D), np.float32)
    for b in range(B):
        od = np.zeros((D, L), np.float32)
        for ci in range(4):
            shard = res.results[4 * b + ci]["out_p"].astype(np.float32)
            for ch in range(4):
                od[ch * 256 + ci * 64: ch * 256 + (ci + 1) * 64] = \
                    shard[ch * 64:(ch + 1) * 64]
        out[b] = od.T
    return out



# revision 8
# speedup vs baseline: 1.0267x; 1.0267x over previous
"""BiMamba (fwd+bwd Mamba + merge) Trainium2 Bass kernel.

Sharding (8 cores): core = batch*4 + dir*2 + e_half.
Each core computes one (batch, direction) pair over 1024 of the 2048 d_inner
channels, in e-partition layout [e_p=128 x 8 tiles, t_free=1024].
bwd cores operate entirely in flipped time (host pre-flips x); the host
un-flips and sums directions after readback.  A single f16 AllReduce merges
the x_proj partials across each e-half pair; a pair ReduceScatter splits the
out_proj partial sum (512 d-rows per core).

Self-contained: hardcodes B=2, L=1024, D=1024, E=2048 (1024/core), N=16,
dt_rank=64, d_conv=4.
"""
import numpy as np

B, L, D = 2, 1024, 1024
E = 2048
EH = 1024            # channels per core (half of E)
N = 16
DTR = 64
K = 4                # d_conv
M_TILES = 8          # e-tiles per core
NB = 8               # n-plane batches
NPB = 2              # planes per batch
PL = L + 2           # plane stride with 2-col zero gap for the batched scan
P1M = 6              # out_proj pass-1 covers m-tiles [0, P1M)

_nc_cache = {}


def _build_nc():
    import concourse.bacc as bacc
    import concourse.mybir as mybir
    from concourse import tile

    f32, f16 = mybir.dt.float32, mybir.dt.float16
    Alu = mybir.AluOpType
    Act = mybir.ActivationFunctionType

    nc = bacc.Bacc("TRN2", target_bir_lowering=False, debug=False, num_devices=8)

    # ---- DRAM I/O ----
    xT_d = nc.dram_tensor("xT", [D, 3 + L], f16, kind="ExternalInput")
    # pre-tiled: [p, m*1024 + kt*128 + e']  (one DMA per m-slab)
    wxiT_d = nc.dram_tensor("wxiT", [128, M_TILES * EH], f16, kind="ExternalInput")
    wzT_d = nc.dram_tensor("wzT", [128, M_TILES * EH], f16, kind="ExternalInput")
    convw_d = nc.dram_tensor("convw", [128, M_TILES * K], f32, kind="ExternalInput")
    convb_d = nc.dram_tensor("convb", [128, M_TILES], f32, kind="ExternalInput")
    xpT_d = nc.dram_tensor("xpT", [EH, 96], f16, kind="ExternalInput")
    dtwT_d = nc.dram_tensor("dtwT", [DTR, EH], f16, kind="ExternalInput")
    dtb_d = nc.dram_tensor("dtb", [128, M_TILES], f32, kind="ExternalInput")
    arate_d = nc.dram_tensor("arate", [128, M_TILES * N], f32, kind="ExternalInput")
    dp_d = nc.dram_tensor("dp", [128, M_TILES], f32, kind="ExternalInput")
    # pre-tiled: [p, dm*1024 + m*128 + d']
    woT_d = nc.dram_tensor("woT", [128, M_TILES * D], f16, kind="ExternalInput")

    dbl_in = nc.dram_tensor("dbl_in", [96, L], f16, kind="Internal")
    dbl_out = nc.dram_tensor("dbl_out", [96, L], f16, kind="Internal")
    oc_in = nc.dram_tensor("oc_in", [D, L], f16, kind="Internal")
    oc_out = nc.dram_tensor("oc_out", [512, L], f16, kind="Internal")
    out_d = nc.dram_tensor("out_p", [512, L], f16, kind="ExternalOutput")

    with tile.TileContext(nc) as tc:
        with tc.tile_pool(name="const", bufs=1) as cpool, \
             tc.tile_pool(name="res", bufs=1) as rpool:
            convw = cpool.tile([128, M_TILES * K], f32)
            convb = cpool.tile([128, M_TILES], f32)
            dtb = cpool.tile([128, M_TILES], f32)
            arate = cpool.tile([128, M_TILES * N], f32)
            dp = cpool.tile([128, M_TILES], f32)
            for t_, d_ in ((convw, convw_d), (convb, convb_d), (dtb, dtb_d),
                           (arate, arate_d), (dp, dp_d)):
                nc.sync.dma_start(t_[:], d_[:])

            xc16 = rpool.tile([128, M_TILES * L], f16)
            sz16 = rpool.tile([128, M_TILES * L], f16)
            g16 = rpool.tile([128, M_TILES * L], f16)
            bca = rpool.tile([128, N * L], f16)
            bcc = rpool.tile([128, N * L], f16)
            dblr16 = rpool.tile([96, L], f16)
            delta16 = rpool.tile([128, M_TILES * L], f16)
            part16 = rpool.tile([128, M_TILES * L], f16)   # out_proj pass-1

            # ---------- Phase A: in_proj matmuls + conv + silu ----------
            with tc.tile_pool(name="pa", bufs=1) as pap, \
                 tc.tile_pool(name="paw", bufs=4) as pwp, \
                 tc.tile_pool(name="pax", bufs=2) as pxp, \
                 tc.tile_pool(name="psA", bufs=2, space="PSUM") as psA:
                xT = pap.tile([128, M_TILES * (3 + L)], f16)
                dma_engs = [nc.sync, nc.scalar, nc.gpsimd]
                for kt in range(M_TILES):
                    dma_engs[kt % 3].dma_start(xT[:, kt * (3 + L):(kt + 1) * (3 + L)],
                                               xT_d[kt * 128:(kt + 1) * 128, :])
                for m in range(M_TILES):
                    wxi = pwp.tile([128, M_TILES * 128], f16, tag="wxi")
                    wz = pwp.tile([128, M_TILES * 128], f16, tag="wz")
                    nc.scalar.dma_start(wxi[:], wxiT_d[:, m * EH:(m + 1) * EH])
                    nc.gpsimd.dma_start(wz[:], wzT_d[:, m * EH:(m + 1) * EH])
                    ps_xi = psA.tile([128, L], f32, tag="xi")
                    ps_z = psA.tile([128, L], f32, tag="z")
                    for kt in range(M_TILES):
                        xk = xT[:, kt * (3 + L):(kt + 1) * (3 + L)]
                        for h in range(2):
                            nc.tensor.matmul(ps_xi[:, h * 512:(h + 1) * 512],
                                             wxi[:, kt * 128:(kt + 1) * 128],
                                             xk[:, 3 + h * 512: 3 + (h + 1) * 512],
                                             start=(kt == 0), stop=(kt == M_TILES - 1))
                            nc.tensor.matmul(ps_z[:, h * 512:(h + 1) * 512],
                                             wz[:, kt * 128:(kt + 1) * 128],
                                             xk[:, 3 + h * 512: 3 + (h + 1) * 512],
                                             start=(kt == 0), stop=(kt == M_TILES - 1))
                    # conv in f16: 4 tensor_scalar taps + add tree (rotating slots)
                    xi16 = pxp.tile([128, 3 + L], f16, tag="xi16")
                    if m < 2:
                        nc.vector.memset(xi16[:, 0:3], 0.0)  # 2 slots, gap stays 0
                    nc.scalar.copy(xi16[:, 3:3 + L], ps_xi[:])
                    def tap(k):
                        t_ = pxp.tile([128, L], f16, tag="ct")
                        nc.vector.tensor_scalar_mul(t_[:], xi16[:, k:k + L],
                                                    convw[:, m * K + k:m * K + k + 1])
                        return t_
                    ta, tb = tap(0), tap(1)
                    s01 = pxp.tile([128, L], f16, tag="cs")
                    nc.vector.tensor_add(s01[:], ta[:], tb[:])
                    tc_, td = tap(2), tap(3)
                    s23 = pxp.tile([128, L], f16, tag="cs")
                    nc.vector.tensor_add(s23[:], tc_[:], td[:])
                    cacc = pxp.tile([128, L], f16, tag="cacc")
                    nc.vector.tensor_add(cacc[:], s01[:], s23[:])
                    nc.scalar.activation(xc16[:, m * L:(m + 1) * L], cacc[:],
                                         Act.Silu, bias=convb[:, m:m + 1])
                    nc.scalar.activation(sz16[:, m * L:(m + 1) * L], ps_z[:], Act.Silu)

            # ---------- Phase B: x_proj partial + single AllReduce + broadcasts ----------
            with tc.tile_pool(name="pb", bufs=1) as pbp, \
                 tc.tile_pool(name="pbw", bufs=2) as pbw, \
                 tc.tile_pool(name="psB", bufs=1, space="PSUM") as psB:
                ps_dbl = psB.tile([96, L], f32)
                for m in range(M_TILES):
                    xp = pbw.tile([128, 96], f16, tag="xp")
                    nc.sync.dma_start(xp[:], xpT_d[m * 128:(m + 1) * 128, :])
                    for h in range(2):
                        nc.tensor.matmul(ps_dbl[:, h * 512:(h + 1) * 512], xp[:],
                                         xc16[:, m * L + h * 512: m * L + (h + 1) * 512],
                                         start=(m == 0), stop=(m == M_TILES - 1))
                cvt16 = pbp.tile([96, L], f16)
                nc.scalar.copy(cvt16[:], ps_dbl[:])
                nc.sync.dma_start(dbl_in[:], cvt16[:])
                nc.gpsimd.collective_compute(
                    "AllReduce", Alu.add,
                    replica_groups=[[0, 1], [2, 3], [4, 5], [6, 7]],
                    ins=[dbl_in[:]], outs=[dbl_out[:]])
                nc.sync.dma_start(dblr16[:], dbl_out[:])
                engs = [nc.sync, nc.scalar, nc.gpsimd]
                for n in range(N):
                    engs[n % 3].dma_start(bca[:, n * L:(n + 1) * L],
                                          dbl_out[64 + n:64 + n + 1, :].broadcast_to([128, L]))
                    engs[(n + 1) % 3].dma_start(bcc[:, n * L:(n + 1) * L],
                                                dbl_out[80 + n:80 + n + 1, :].broadcast_to([128, L]))

            # ---------- Phase C: delta (batched), dA planes, scan, y ----------
            with tc.tile_pool(name="pc", bufs=2) as pcp, \
                 tc.tile_pool(name="pc1", bufs=2) as pc1, \
                 tc.tile_pool(name="pcw", bufs=2) as pcw, \
                 tc.tile_pool(name="psC", bufs=2, space="PSUM") as psC, \
                 tc.tile_pool(name="psD", bufs=2, space="PSUM") as psD:
                # all dt matmuls + softplus upfront, batched per act table:
                # ee = exp(dt+bias) into part16 scratch, then delta = ln(1+ee)
                for m in range(M_TILES):
                    dtw = pcp.tile([DTR, 128], f16, tag="dtw")
                    nc.sync.dma_start(dtw[:], dtwT_d[:, m * 128:(m + 1) * 128])
                    ps_dt = psC.tile([128, L], f32, tag="dt")
                    for h in range(2):
                        nc.tensor.matmul(ps_dt[:, h * 512:(h + 1) * 512], dtw[:],
                                         dblr16[0:DTR, h * 512:(h + 1) * 512],
                                         start=True, stop=True)
                    nc.scalar.activation(part16[:, m * L:(m + 1) * L], ps_dt[:],
                                         Act.Exp, bias=dtb[:, m:m + 1])
                for m in range(M_TILES):
                    nc.scalar.activation(delta16[:, m * L:(m + 1) * L],
                                         part16[:, m * L:(m + 1) * L],
                                         Act.Ln, bias=1.0)

                bca3 = bca[:].rearrange("p (n l) -> p n l", l=L)
                bcc3 = bcc[:].rearrange("p (n l) -> p n l", l=L)
                for m in range(M_TILES):
                    u16 = pcp.tile([128, L], f16, tag="u16")
                    nc.vector.tensor_mul(u16[:], delta16[:, m * L:(m + 1) * L],
                                         xc16[:, m * L:(m + 1) * L])
                    yacc = None
                    for nb in range(NB):
                        dA = pcp.tile([128, NPB * PL], f32, tag="dA")
                        dBu = pcp.tile([128, NPB * PL], f16, tag="dBu")
                        for j in range(NPB):
                            n = nb * NPB + j
                            nc.scalar.activation(dA[:, j * PL:j * PL + L],
                                                 delta16[:, m * L:(m + 1) * L],
                                                 Act.Exp,
                                                 scale=arate[:, m * N + n:m * N + n + 1])
                        dA3 = dA[:].rearrange("p (n l) -> p n l", l=PL)
                        dBu3 = dBu[:].rearrange("p (n l) -> p n l", l=PL)
                        if m == 0 and nb < 2:
                            # gap columns stay 0 across slot reuse (2 slots/tag)
                            nc.vector.memset(dA3[:, :, L:PL], 0.0)
                            nc.vector.memset(dBu3[:, :, L:PL], 0.0)
                        nc.vector.tensor_mul(
                            dBu3[:, :, 0:L],
                            u16[:, None, :].broadcast_to([128, NPB, L]),
                            bca3[:, nb * NPB:(nb + 1) * NPB, :])
                        h4 = pcp.tile([128, NPB * PL], f16, tag="h4")
                        nc.vector.tensor_tensor_scan(h4[:], dA[:], dBu[:], 0.0,
                                                     Alu.mult, Alu.add)
                        h43 = h4[:].rearrange("p (n l) -> p n l", l=PL)
                        prod = pcp.tile([128, NPB * PL], f16, tag="dBu")
                        prod3 = prod[:].rearrange("p (n l) -> p n l", l=PL)
                        nc.vector.tensor_mul(prod3[:, :, 0:L], h43[:, :, 0:L],
                                             bcc3[:, nb * NPB:(nb + 1) * NPB, :])
                        if nb == 0:
                            # seed yacc with the dp skip term + first pair
                            xdp = pc1.tile([128, L], f16, tag="xdp")
                            nc.vector.tensor_scalar_mul(
                                xdp[:], xc16[:, m * L:(m + 1) * L], dp[:, m:m + 1])
                            yp = pc1.tile([128, L], f16, tag="ypp")
                            nc.vector.tensor_add(yp[:], prod[:, 0:L],
                                                 prod[:, PL:PL + L])
                            yacc = pc1.tile([128, L], f16, tag="yac")
                            nc.vector.tensor_add(yacc[:], xdp[:], yp[:])
                        else:
                            yp = pc1.tile([128, L], f16, tag="ypp")
                            nc.vector.tensor_add(yp[:], prod[:, 0:L],
                                                 prod[:, PL:PL + L])
                            ynew = pc1.tile([128, L], f16, tag="yac")
                            nc.vector.tensor_add(ynew[:], yacc[:], yp[:])
                            yacc = ynew
                    nc.vector.tensor_mul(g16[:, m * L:(m + 1) * L], yacc[:],
                                         sz16[:, m * L:(m + 1) * L])

                    # ---- out_proj pass-1 once g16[0:P1M] ready ----
                    if m == P1M - 1:
                        for dm in range(M_TILES):
                            wo = pcw.tile([128, M_TILES * 128], f16, tag="wo")
                            nc.sync.dma_start(wo[:], woT_d[:, dm * D:(dm + 1) * D])
                            ps_o = psD.tile([128, L], f32, tag="o")
                            for mm in range(P1M):
                                for h in range(2):
                                    nc.tensor.matmul(
                                        ps_o[:, h * 512:(h + 1) * 512],
                                        wo[:, mm * 128:(mm + 1) * 128],
                                        g16[:, mm * L + h * 512: mm * L + (h + 1) * 512],
                                        start=(mm == 0), stop=(mm == P1M - 1))
                            nc.scalar.copy(part16[:, dm * L:(dm + 1) * L], ps_o[:])

                # ---- out_proj pass-2 (m = P1M..7) + pair ReduceScatter ----
                for dm in range(M_TILES):
                    wo = pcw.tile([128, M_TILES * 128], f16, tag="wo")
                    nc.sync.dma_start(wo[:], woT_d[:, dm * D:(dm + 1) * D])
                    ps_o = psD.tile([128, L], f32, tag="o")
                    for mm in range(P1M, M_TILES):
                        for h in range(2):
                            nc.tensor.matmul(
                                ps_o[:, h * 512:(h + 1) * 512],
                                wo[:, mm * 128:(mm + 1) * 128],
                                g16[:, mm * L + h * 512: mm * L + (h + 1) * 512],
                                start=(mm == P1M), stop=(mm == M_TILES - 1))
                    oc16 = pcp.tile([128, L], f16, tag="oc16")
                    nc.vector.tensor_add(oc16[:], part16[:, dm * L:(dm + 1) * L], ps_o[:])
                    nc.sync.dma_start(oc_in[dm * 128:(dm + 1) * 128, :], oc16[:])
                    if dm % 2 == 1:
                        # pair RS: group-rank r gets rows [ch*256+r*128, +128)
                        ch = dm // 2
                        nc.gpsimd.collective_compute(
                            "ReduceScatter", Alu.add,
                            replica_groups=[[0, 1], [2, 3], [4, 5], [6, 7]],
                            ins=[oc_in[ch * 256:(ch + 1) * 256, :]],
                            outs=[oc_out[ch * 128:(ch + 1) * 128, :]])
                        nc.sync.dma_start(out_d[ch * 128:(ch + 1) * 128, :],
                                          oc_out[ch * 128:(ch + 1) * 128, :])

    nc.compile()
    return nc


def _host_prep(inputs):
    """Build the 8 per-core input maps from the full problem inputs."""
    x = np.asarray(inputs["x"], np.float32)
    merge_w = np.asarray(inputs["merge_w"], np.float32)
    in_maps = []
    for b in range(B):
        for di, pre in enumerate(("fwd", "bwd")):
            p = {k: np.asarray(inputs[f"{pre}_{k}"], np.float32)
                 for k in ("in_proj", "conv_w", "conv_b", "x_proj", "dt_w",
                           "dt_b", "A_log", "D", "out_proj")}
            xb = x[b]
            if di == 1:
                xb = xb[::-1]
            xTp = np.concatenate([np.zeros((D, 3), np.float32), xb.T], axis=1)
            A = -np.exp(p["A_log"])                       # (E, N)
            W = merge_w[:, di * D:(di + 1) * D] @ p["out_proj"]   # (D, E)
            def pack_lhsT(wT):
                # (D, EH) -> [p, m*1024 + kt*128 + e']
                return np.ascontiguousarray(
                    wT.reshape(M_TILES, 128, M_TILES, 128).transpose(1, 2, 0, 3)
                    .reshape(128, M_TILES * EH))

            for half in range(2):
                sl = slice(half * EH, (half + 1) * EH)
                wxiT = pack_lhsT(p["in_proj"][:E][sl].T)
                wzT = pack_lhsT(p["in_proj"][E:][sl].T)
                convw = p["conv_w"][sl].reshape(M_TILES, 128, K).transpose(1, 0, 2).reshape(128, M_TILES * K)
                convb = p["conv_b"][sl].reshape(M_TILES, 128).T
                xpT = p["x_proj"][:, sl].T                # (EH, 96)
                dtwT = p["dt_w"][sl].T                    # (DTR, EH)
                dtb = p["dt_b"][sl].reshape(M_TILES, 128).T
                arate = A[sl].reshape(M_TILES, 128, N).transpose(1, 0, 2).reshape(128, M_TILES * N)
                dpv = p["D"][sl].reshape(M_TILES, 128).T
                woT = pack_lhsT(W[:, sl].T)               # (EH, D) pre-tiled
                in_maps.append({
                    "xT": xTp.astype(np.float16),
                    "wxiT": wxiT.astype(np.float16),
                    "wzT": wzT.astype(np.float16),
                    "convw": np.ascontiguousarray(convw, np.float32),
                    "convb": np.ascontiguousarray(convb, np.float32),
                    "xpT": xpT.astype(np.float16),
                    "dtwT": dtwT.astype(np.float16),
                    "dtb": np.ascontiguousarray(dtb, np.float32),
                    "arate": np.ascontiguousarray(arate, np.float32),
                    "dp": np.ascontiguousarray(dpv, np.float32),
                    "woT": woT.astype(np.float16),
                })
    return in_maps


def _ensure_neuron_platform():
    """If a caller pinned jax to cpu, re-point it at the neuron/axon PJRT
    platform so run_bass_kernel_spmd sees the 8 NeuronCores."""
    import jax
    try:
        if len(jax.devices()) >= 8 and jax.devices()[0].platform != "cpu":
            return
    except Exception:
        pass
    for plat in ("axon", "neuron"):
        try:
            jax.config.update("jax_platforms", plat)
            if len(jax.devices()) >= 8:
                return
        except Exception:
            continue


def kernel(**inputs):
    _ensure_neuron_platform()
    from concourse.bass_utils import run_bass_kernel_spmd
    if "nc" not in _nc_cache:
        _nc_cache["nc"] = _build_nc()
    nc = _nc_cache["nc"]
    in_maps = _host_prep(inputs)
    res = run_bass_kernel_spmd(nc, in_maps, core_ids=list(range(8)))
    _nc_cache["last_results"] = res
    # Stitch: core 4b+2*di+r holds, for chunk ch in 0..3, global d-rows
    # [ch*256 + r*128, +128) at out_p[ch*128:(ch+1)*128]; bwd cores are in
    # flipped time.
    out = np.zeros((B, L, D), np.float32)
    for b in range(B):
        acc = np.zeros((D, L), np.float32)
        for di in range(2):
            od = np.zeros((D, L), np.float32)
            for r in range(2):
                shard = res.results[4 * b + 2 * di + r]["out_p"].astype(np.float32)
                for ch in range(4):
                    od[ch * 256 + r * 128: ch * 256 + (r + 1) * 128] = \
                        shard[ch * 128:(ch + 1) * 128]
            if di == 1:
                od = od[:, ::-1]
            acc += od
        out[b] = acc.T
    return out


# revision 10
# speedup vs baseline: 1.1612x; 1.1310x over previous
"""BiMamba (fwd+bwd Mamba + merge) Trainium2 Bass kernel.

Sharding (8 cores): core = batch*4 + dir*2 + e_half.
Each core computes one (batch, direction) pair over 1024 of the 2048 d_inner
channels, in e-partition layout [e_p=128 x 8 tiles, t_free=1024].
bwd cores operate entirely in flipped time (host pre-flips x); the host
un-flips and sums directions after readback.  A single f16 AllReduce merges
the x_proj partials across each e-half pair; a pair ReduceScatter splits the
out_proj partial sum (512 d-rows per core).

Self-contained: hardcodes B=2, L=1024, D=1024, E=2048 (1024/core), N=16,
dt_rank=64, d_conv=4.
"""
import numpy as np

B, L, D = 2, 1024, 1024
E = 2048
EH = 1024            # channels per core (half of E)
N = 16
DTR = 64
K = 4                # d_conv
M_TILES = 8          # e-tiles per core
NB = 8               # n-plane batches
NPB = 2              # planes per batch
PL = L + 2           # plane stride with 2-col zero gap for the batched scan
P1M = 6              # out_proj pass-1 covers m-tiles [0, P1M)

_nc_cache = {}


def _build_nc():
    import concourse.bacc as bacc
    import concourse.mybir as mybir
    from concourse import tile

    f32, f16 = mybir.dt.float32, mybir.dt.float16
    Alu = mybir.AluOpType
    Act = mybir.ActivationFunctionType

    nc = bacc.Bacc("TRN2", target_bir_lowering=False, debug=False, num_devices=8)

    # ---- DRAM I/O ----
    xT_d = nc.dram_tensor("xT", [D, 3 + L], f16, kind="ExternalInput")
    # pre-tiled: [p, m*1024 + kt*128 + e']  (one DMA per m-slab)
    wxiT_d = nc.dram_tensor("wxiT", [128, M_TILES * EH], f16, kind="ExternalInput")
    wzT_d = nc.dram_tensor("wzT", [128, M_TILES * EH], f16, kind="ExternalInput")
    convw_d = nc.dram_tensor("convw", [128, M_TILES * K], f32, kind="ExternalInput")
    convb_d = nc.dram_tensor("convb", [128, M_TILES], f32, kind="ExternalInput")
    xpT_d = nc.dram_tensor("xpT", [EH, 96], f16, kind="ExternalInput")
    dtwT_d = nc.dram_tensor("dtwT", [DTR, EH], f16, kind="ExternalInput")
    dtb_d = nc.dram_tensor("dtb", [128, M_TILES], f32, kind="ExternalInput")
    arate_d = nc.dram_tensor("arate", [128, M_TILES * N], f32, kind="ExternalInput")
    dp_d = nc.dram_tensor("dp", [128, M_TILES], f32, kind="ExternalInput")
    # pre-tiled: [p, dm*1024 + m*128 + d']
    woT_d = nc.dram_tensor("woT", [128, M_TILES * D], f16, kind="ExternalInput")
    ident_d = nc.dram_tensor("ident", [128, 128], f16, kind="ExternalInput")

    dbl_in = nc.dram_tensor("dbl_in", [96, L], f16, kind="Internal")
    dbl_out = nc.dram_tensor("dbl_out", [96, L], f16, kind="Internal")
    out_d = nc.dram_tensor("out_p", [D, L], f16, kind="ExternalOutput")

    with tile.TileContext(nc) as tc:
        with tc.tile_pool(name="const", bufs=1) as cpool, \
             tc.tile_pool(name="res", bufs=1) as rpool:
            convw = cpool.tile([128, M_TILES * K], f32)
            convb = cpool.tile([128, M_TILES], f32)
            dtb = cpool.tile([128, M_TILES], f32)
            arate = cpool.tile([128, M_TILES * N], f32)
            dp = cpool.tile([128, M_TILES], f32)
            ident = cpool.tile([128, 128], f16)
            dtwT = cpool.tile([DTR, M_TILES * 128], f16)
            for t_, d_ in ((convw, convw_d), (convb, convb_d), (dtb, dtb_d),
                           (arate, arate_d), (dp, dp_d), (ident, ident_d),
                           (dtwT, dtwT_d)):
                nc.sync.dma_start(t_[:], d_[:])

            xc16 = rpool.tile([128, M_TILES * L], f16)
            sz16 = rpool.tile([128, M_TILES * L], f16)
            g16 = rpool.tile([128, M_TILES * L], f16)
            bca = rpool.tile([128, N * L], f16)
            bcc = rpool.tile([128, N * L], f16)
            dblr16 = rpool.tile([96, L], f16)
            part16 = rpool.tile([128, M_TILES * L], f16)   # out_proj pass-1

            # ---------- Phase A: in_proj matmuls + conv + silu ----------
            with tc.tile_pool(name="pa", bufs=1) as pap, \
                 tc.tile_pool(name="paw", bufs=4) as pwp, \
                 tc.tile_pool(name="pax", bufs=2) as pxp, \
                 tc.tile_pool(name="psA", bufs=2, space="PSUM") as psA:
                xT = pap.tile([128, M_TILES * (3 + L)], f16)
                dma_engs = [nc.sync, nc.scalar, nc.gpsimd]
                for kt in range(M_TILES):
                    dma_engs[kt % 3].dma_start(xT[:, kt * (3 + L):(kt + 1) * (3 + L)],
                                               xT_d[kt * 128:(kt + 1) * 128, :])
                for m in range(M_TILES):
                    wxi = pwp.tile([128, M_TILES * 128], f16, tag="wxi")
                    wz = pwp.tile([128, M_TILES * 128], f16, tag="wz")
                    nc.scalar.dma_start(wxi[:], wxiT_d[:, m * EH:(m + 1) * EH])
                    nc.gpsimd.dma_start(wz[:], wzT_d[:, m * EH:(m + 1) * EH])
                    ps_xi = psA.tile([128, L], f32, tag="xi")
                    ps_z = psA.tile([128, L], f32, tag="z")
                    for kt in range(M_TILES):
                        xk = xT[:, kt * (3 + L):(kt + 1) * (3 + L)]
                        for h in range(2):
                            nc.tensor.matmul(ps_xi[:, h * 512:(h + 1) * 512],
                                             wxi[:, kt * 128:(kt + 1) * 128],
                                             xk[:, 3 + h * 512: 3 + (h + 1) * 512],
                                             start=(kt == 0), stop=(kt == M_TILES - 1))
                            nc.tensor.matmul(ps_z[:, h * 512:(h + 1) * 512],
                                             wz[:, kt * 128:(kt + 1) * 128],
                                             xk[:, 3 + h * 512: 3 + (h + 1) * 512],
                                             start=(kt == 0), stop=(kt == M_TILES - 1))
                    # conv in f16: 4 tensor_scalar taps + add tree (rotating slots)
                    xi16 = pxp.tile([128, 3 + L], f16, tag="xi16")
                    if m < 2:
                        nc.vector.memset(xi16[:, 0:3], 0.0)  # 2 slots, gap stays 0
                    nc.scalar.copy(xi16[:, 3:3 + L], ps_xi[:])
                    def tap(k):
                        t_ = pxp.tile([128, L], f16, tag="ct")
                        nc.vector.tensor_scalar_mul(t_[:], xi16[:, k:k + L],
                                                    convw[:, m * K + k:m * K + k + 1])
                        return t_
                    ta, tb = tap(0), tap(1)
                    s01 = pxp.tile([128, L], f16, tag="cs")
                    nc.vector.tensor_add(s01[:], ta[:], tb[:])
                    tc_, td = tap(2), tap(3)
                    s23 = pxp.tile([128, L], f16, tag="cs")
                    nc.vector.tensor_add(s23[:], tc_[:], td[:])
                    cacc = pxp.tile([128, L], f16, tag="cacc")
                    nc.vector.tensor_add(cacc[:], s01[:], s23[:])
                    nc.scalar.activation(xc16[:, m * L:(m + 1) * L], cacc[:],
                                         Act.Silu, bias=convb[:, m:m + 1])
                    nc.scalar.activation(sz16[:, m * L:(m + 1) * L], ps_z[:], Act.Silu)

            # ---------- Phase B: x_proj partial + single AllReduce + broadcasts ----------
            with tc.tile_pool(name="pb", bufs=1) as pbp, \
                 tc.tile_pool(name="pbw", bufs=2) as pbw, \
                 tc.tile_pool(name="psB", bufs=1, space="PSUM") as psB:
                ps_dbl = psB.tile([96, L], f32)
                for m in range(M_TILES):
                    xp = pbw.tile([128, 96], f16, tag="xp")
                    nc.sync.dma_start(xp[:], xpT_d[m * 128:(m + 1) * 128, :])
                    for h in range(2):
                        nc.tensor.matmul(ps_dbl[:, h * 512:(h + 1) * 512], xp[:],
                                         xc16[:, m * L + h * 512: m * L + (h + 1) * 512],
                                         start=(m == 0), stop=(m == M_TILES - 1))
                cvt16 = pbp.tile([96, L], f16)
                nc.scalar.copy(cvt16[:], ps_dbl[:])
                nc.sync.dma_start(dbl_in[:], cvt16[:])
                nc.gpsimd.collective_compute(
                    "AllReduce", Alu.add,
                    replica_groups=[[0, 1], [2, 3], [4, 5], [6, 7]],
                    ins=[dbl_in[:]], outs=[dbl_out[:]])
                nc.sync.dma_start(dblr16[:], dbl_out[:])
                engs = [nc.sync, nc.scalar, nc.gpsimd]
                for n in range(N):
                    engs[n % 3].dma_start(bca[:, n * L:(n + 1) * L],
                                          dbl_out[64 + n:64 + n + 1, :].broadcast_to([128, L]))
                    engs[(n + 1) % 3].dma_start(bcc[:, n * L:(n + 1) * L],
                                                dbl_out[80 + n:80 + n + 1, :].broadcast_to([128, L]))

            # ---------- Phase C: delta (batched), dA planes, scan, y ----------
            with tc.tile_pool(name="pc", bufs=2) as pcp, \
                 tc.tile_pool(name="pc1", bufs=2) as pc1, \
                 tc.tile_pool(name="pcw", bufs=2) as pcw, \
                 tc.tile_pool(name="psC", bufs=1, space="PSUM") as psC, \
                 tc.tile_pool(name="psY", bufs=2, space="PSUM") as psY, \
                 tc.tile_pool(name="psD", bufs=1, space="PSUM") as psD:
                bca3 = bca[:].rearrange("p (n l) -> p n l", l=L)
                bcc3 = bcc[:].rearrange("p (n l) -> p n l", l=L)
                for m in range(M_TILES):
                    # delta = softplus(dt@dtw + dtb), just-in-time per m
                    ps_dt = psC.tile([128, L], f32, tag="dt")
                    for h in range(2):
                        nc.tensor.matmul(ps_dt[:, h * 512:(h + 1) * 512],
                                         dtwT[:, m * 128:(m + 1) * 128],
                                         dblr16[0:DTR, h * 512:(h + 1) * 512],
                                         start=True, stop=True)
                    ee16 = pcp.tile([128, L], f16, tag="ee16")
                    nc.scalar.activation(ee16[:], ps_dt[:], Act.Exp,
                                         bias=dtb[:, m:m + 1])
                    delta16 = pcp.tile([128, L], f16, tag="d16")
                    nc.scalar.activation(delta16[:], ee16[:], Act.Ln, bias=1.0)
                    u16 = pcp.tile([128, L], f16, tag="u16")
                    nc.vector.tensor_mul(u16[:], delta16[:],
                                         xc16[:, m * L:(m + 1) * L])
                    ps_y = psY.tile([128, L], f32, tag="ysum")
                    for nb in range(NB):
                        dA = pcp.tile([128, NPB * PL], f32, tag="dA")
                        dBu = pcp.tile([128, NPB * PL], f16, tag="dBu")
                        for j in range(NPB):
                            n = nb * NPB + j
                            nc.scalar.activation(dA[:, j * PL:j * PL + L],
                                                 delta16[:], Act.Exp,
                                                 scale=arate[:, m * N + n:m * N + n + 1])
                        dA3 = dA[:].rearrange("p (n l) -> p n l", l=PL)
                        dBu3 = dBu[:].rearrange("p (n l) -> p n l", l=PL)
                        if m == 0 and nb < 2:
                            # gap columns stay 0 across slot reuse (2 slots/tag)
                            nc.vector.memset(dA3[:, :, L:PL], 0.0)
                            nc.vector.memset(dBu3[:, :, L:PL], 0.0)
                        nc.vector.tensor_mul(
                            dBu3[:, :, 0:L],
                            u16[:, None, :].broadcast_to([128, NPB, L]),
                            bca3[:, nb * NPB:(nb + 1) * NPB, :])
                        h4 = pcp.tile([128, NPB * PL], f16, tag="h4")
                        nc.vector.tensor_tensor_scan(h4[:], dA[:], dBu[:], 0.0,
                                                     Alu.mult, Alu.add)
                        h43 = h4[:].rearrange("p (n l) -> p n l", l=PL)
                        prod = pcp.tile([128, NPB * PL], f16, tag="dBu")
                        prod3 = prod[:].rearrange("p (n l) -> p n l", l=PL)
                        nc.vector.tensor_mul(prod3[:, :, 0:L], h43[:, :, 0:L],
                                             bcc3[:, nb * NPB:(nb + 1) * NPB, :])
                        # sum the 16 n-planes on the tensor engine (identity
                        # matmul accumulating into PSUM)
                        for j in range(NPB):
                            for h in range(2):
                                nc.tensor.matmul(
                                    ps_y[:, h * 512:(h + 1) * 512], ident[:],
                                    prod[:, j * PL + h * 512: j * PL + (h + 1) * 512],
                                    start=(nb == 0 and j == 0),
                                    stop=(nb == NB - 1 and j == NPB - 1))
                    xdp = pc1.tile([128, L], f16, tag="xdp")
                    nc.vector.tensor_scalar_mul(
                        xdp[:], xc16[:, m * L:(m + 1) * L], dp[:, m:m + 1])
                    ys16 = pc1.tile([128, L], f16, tag="ys")
                    nc.vector.tensor_add(ys16[:], ps_y[:], xdp[:])
                    nc.vector.tensor_mul(g16[:, m * L:(m + 1) * L], ys16[:],
                                         sz16[:, m * L:(m + 1) * L])

                    # ---- out_proj pass-1 once g16[0:P1M] ready ----
                    if m == P1M - 1:
                        for dm in range(M_TILES):
                            wo = pcw.tile([128, M_TILES * 128], f16, tag="wo")
                            nc.sync.dma_start(wo[:], woT_d[:, dm * D:(dm + 1) * D])
                            ps_o = psD.tile([128, L], f32, tag="o")
                            for mm in range(P1M):
                                for h in range(2):
                                    nc.tensor.matmul(
                                        ps_o[:, h * 512:(h + 1) * 512],
                                        wo[:, mm * 128:(mm + 1) * 128],
                                        g16[:, mm * L + h * 512: mm * L + (h + 1) * 512],
                                        start=(mm == 0), stop=(mm == P1M - 1))
                            nc.scalar.copy(part16[:, dm * L:(dm + 1) * L], ps_o[:])

                # ---- out_proj pass-2 (m = P1M..7) + pair ReduceScatter ----
                for dm in range(M_TILES):
                    wo = pcw.tile([128, M_TILES * 128], f16, tag="wo")
                    nc.sync.dma_start(wo[:], woT_d[:, dm * D:(dm + 1) * D])
                    ps_o = psD.tile([128, L], f32, tag="o")
                    for mm in range(P1M, M_TILES):
                        for h in range(2):
                            nc.tensor.matmul(
                                ps_o[:, h * 512:(h + 1) * 512],
                                wo[:, mm * 128:(mm + 1) * 128],
                                g16[:, mm * L + h * 512: mm * L + (h + 1) * 512],
                                start=(mm == P1M), stop=(mm == M_TILES - 1))
                    oc16 = pcp.tile([128, L], f16, tag="oc16")
                    nc.vector.tensor_add(oc16[:], part16[:, dm * L:(dm + 1) * L], ps_o[:])
                    nc.sync.dma_start(out_d[dm * 128:(dm + 1) * 128, :], oc16[:])

    nc.compile()
    return nc


def _host_prep(inputs):
    """Build the 8 per-core input maps from the full problem inputs."""
    x = np.asarray(inputs["x"], np.float32)
    merge_w = np.asarray(inputs["merge_w"], np.float32)
    in_maps = []
    for b in range(B):
        for di, pre in enumerate(("fwd", "bwd")):
            p = {k: np.asarray(inputs[f"{pre}_{k}"], np.float32)
                 for k in ("in_proj", "conv_w", "conv_b", "x_proj", "dt_w",
                           "dt_b", "A_log", "D", "out_proj")}
            xb = x[b]
            if di == 1:
                xb = xb[::-1]
            xTp = np.concatenate([np.zeros((D, 3), np.float32), xb.T], axis=1)
            A = -np.exp(p["A_log"])                       # (E, N)
            W = merge_w[:, di * D:(di + 1) * D] @ p["out_proj"]   # (D, E)
            def pack_lhsT(wT):
                # (D, EH) -> [p, m*1024 + kt*128 + e']
                return np.ascontiguousarray(
                    wT.reshape(M_TILES, 128, M_TILES, 128).transpose(1, 2, 0, 3)
                    .reshape(128, M_TILES * EH))

            for half in range(2):
                sl = slice(half * EH, (half + 1) * EH)
                wxiT = pack_lhsT(p["in_proj"][:E][sl].T)
                wzT = pack_lhsT(p["in_proj"][E:][sl].T)
                convw = p["conv_w"][sl].reshape(M_TILES, 128, K).transpose(1, 0, 2).reshape(128, M_TILES * K)
                convb = p["conv_b"][sl].reshape(M_TILES, 128).T
                xpT = p["x_proj"][:, sl].T                # (EH, 96)
                dtwT = p["dt_w"][sl].T                    # (DTR, EH)
                dtb = p["dt_b"][sl].reshape(M_TILES, 128).T
                arate = A[sl].reshape(M_TILES, 128, N).transpose(1, 0, 2).reshape(128, M_TILES * N)
                dpv = p["D"][sl].reshape(M_TILES, 128).T
                woT = pack_lhsT(W[:, sl].T)               # (EH, D) pre-tiled
                in_maps.append({
                    "xT": xTp.astype(np.float16),
                    "wxiT": wxiT.astype(np.float16),
                    "wzT": wzT.astype(np.float16),
                    "convw": np.ascontiguousarray(convw, np.float32),
                    "convb": np.ascontiguousarray(convb, np.float32),
                    "xpT": xpT.astype(np.float16),
                    "dtwT": dtwT.astype(np.float16),
                    "dtb": np.ascontiguousarray(dtb, np.float32),
                    "arate": np.ascontiguousarray(arate, np.float32),
                    "dp": np.ascontiguousarray(dpv, np.float32),
                    "woT": woT.astype(np.float16),
                    "ident": np.eye(128, dtype=np.float16),
                })
    return in_maps


def _ensure_neuron_platform():
    """If a caller pinned jax to cpu, re-point it at the neuron/axon PJRT
    platform so run_bass_kernel_spmd sees the 8 NeuronCores."""
    import jax
    try:
        if len(jax.devices()) >= 8 and jax.devices()[0].platform != "cpu":
            return
    except Exception:
        pass
    for plat in ("axon", "neuron"):
        try:
            jax.config.update("jax_platforms", plat)
            if len(jax.devices()) >= 8:
                return
        except Exception:
            continue


def kernel(**inputs):
    _ensure_neuron_platform()
    from concourse.bass_utils import run_bass_kernel_spmd
    if "nc" not in _nc_cache:
        _nc_cache["nc"] = _build_nc()
    nc = _nc_cache["nc"]
    in_maps = _host_prep(inputs)
    res = run_bass_kernel_spmd(nc, in_maps, core_ids=list(range(8)))
    _nc_cache["last_results"] = res
    # Stitch: each core returns its full [D, L] out_proj partial (own e-half,
    # merge_w folded); host sums halves + directions, un-flipping bwd time.
    out = np.zeros((B, L, D), np.float32)
    for b in range(B):
        acc = np.zeros((D, L), np.float32)
        for di in range(2):
            od = (res.results[4 * b + 2 * di + 0]["out_p"].astype(np.float32) +
                  res.results[4 * b + 2 * di + 1]["out_p"].astype(np.float32))
            if di == 1:
                od = od[:, ::-1]
            acc += od
        out[b] = acc.T
    return out


# revision 14
# speedup vs baseline: 1.1764x; 1.0131x over previous
"""BiMamba (fwd+bwd Mamba + merge) Trainium2 Bass kernel.

Sharding (8 cores): core = batch*4 + dir*2 + e_half.
Each core computes one (batch, direction) pair over 1024 of the 2048 d_inner
channels, in e-partition layout [e_p=128 x 8 tiles, t_free=1024].
bwd cores operate entirely in flipped time (host pre-flips x); the host
un-flips and sums directions after readback.  A single f16 AllReduce merges
the x_proj partials across each e-half pair; a pair ReduceScatter splits the
out_proj partial sum (512 d-rows per core).

Self-contained: hardcodes B=2, L=1024, D=1024, E=2048 (1024/core), N=16,
dt_rank=64, d_conv=4.
"""
import numpy as np

B, L, D = 2, 1024, 1024
E = 2048
EH = 1024            # channels per core (half of E)
N = 16
DTR = 64
K = 4                # d_conv
M_TILES = 8          # e-tiles per core
NB = 8               # n-plane batches
NPB = 2              # planes per batch
PL = L + 2           # plane stride with 2-col zero gap for the batched scan
P1M = 7              # out_proj pass-1 covers m-tiles [0, P1M)

_nc_cache = {}


def _build_nc():
    import concourse.bacc as bacc
    import concourse.mybir as mybir
    from concourse import tile

    f32, f16 = mybir.dt.float32, mybir.dt.float16
    Alu = mybir.AluOpType
    Act = mybir.ActivationFunctionType

    nc = bacc.Bacc("TRN2", target_bir_lowering=False, debug=False, num_devices=8)

    # ---- DRAM I/O ----
    xT_d = nc.dram_tensor("xT", [D, 3 + L], f16, kind="ExternalInput")
    # pre-tiled: [p, m*1024 + kt*128 + e']  (one DMA per m-slab)
    wxiT_d = nc.dram_tensor("wxiT", [128, M_TILES * EH], f16, kind="ExternalInput")
    wzT_d = nc.dram_tensor("wzT", [128, M_TILES * EH], f16, kind="ExternalInput")
    convw_d = nc.dram_tensor("convw", [128, M_TILES * K], f32, kind="ExternalInput")
    convb_d = nc.dram_tensor("convb", [128, M_TILES], f32, kind="ExternalInput")
    xpT_d = nc.dram_tensor("xpT", [EH, 96], f16, kind="ExternalInput")
    dtwT_d = nc.dram_tensor("dtwT", [DTR, EH], f16, kind="ExternalInput")
    dtb_d = nc.dram_tensor("dtb", [128, M_TILES], f32, kind="ExternalInput")
    arate_d = nc.dram_tensor("arate", [128, M_TILES * N], f32, kind="ExternalInput")
    dp_d = nc.dram_tensor("dp", [128, M_TILES], f32, kind="ExternalInput")
    # pre-tiled: [p, dm*1024 + m*128 + d']
    woT_d = nc.dram_tensor("woT", [128, M_TILES * D], f16, kind="ExternalInput")
    ident_d = nc.dram_tensor("ident", [128, 128], f16, kind="ExternalInput")
    sel32_d = nc.dram_tensor("sel32", [32, 32 * 128], f16, kind="ExternalInput")

    dbl_in = nc.dram_tensor("dbl_in", [96, L], f16, kind="Internal")
    dbl_out = nc.dram_tensor("dbl_out", [96, L], f16, kind="Internal")
    out_d = nc.dram_tensor("out_p", [D, L], f16, kind="ExternalOutput")

    with tile.TileContext(nc) as tc:
        with tc.tile_pool(name="const", bufs=1) as cpool, \
             tc.tile_pool(name="res", bufs=1) as rpool:
            convw = cpool.tile([128, M_TILES * K], f32)
            convb = cpool.tile([128, M_TILES], f32)
            dtb = cpool.tile([128, M_TILES], f32)
            arate = cpool.tile([128, M_TILES * N], f32)
            dp = cpool.tile([128, M_TILES], f32)
            ident = cpool.tile([128, 128], f16)
            # one-hot selector bank: sel[s, n*128+p] = (s == n), n,s in 0..31
            sel32 = cpool.tile([32, 32 * 128], f16)
            nc.scalar.dma_start(sel32[:], sel32_d[:])
            dtwT = cpool.tile([DTR, M_TILES * 128], f16)
            for t_, d_ in ((convw, convw_d), (convb, convb_d), (dtb, dtb_d),
                           (arate, arate_d), (dp, dp_d), (ident, ident_d),
                           (dtwT, dtwT_d)):
                nc.sync.dma_start(t_[:], d_[:])

            xc16 = rpool.tile([128, M_TILES * L], f16)
            sz16 = rpool.tile([128, M_TILES * L], f16)
            g16 = rpool.tile([128, M_TILES * L], f16)
            bca = rpool.tile([128, N * L], f16)
            bcc = rpool.tile([128, N * L], f16)
            dblr16 = rpool.tile([96, L], f16)
            part16 = rpool.tile([128, M_TILES * L], f16)   # out_proj pass-1

            # ---------- Phase A: in_proj matmuls + conv + silu ----------
            with tc.tile_pool(name="pa", bufs=1) as pap, \
                 tc.tile_pool(name="paw", bufs=4) as pwp, \
                 tc.tile_pool(name="pax", bufs=2) as pxp, \
                 tc.tile_pool(name="psA", bufs=2, space="PSUM") as psA:
                xT = pap.tile([128, M_TILES * (3 + L)], f16)
                dma_engs = [nc.sync, nc.scalar, nc.gpsimd]
                for kt in range(M_TILES):
                    dma_engs[kt % 3].dma_start(xT[:, kt * (3 + L):(kt + 1) * (3 + L)],
                                               xT_d[kt * 128:(kt + 1) * 128, :])
                for m in range(M_TILES):
                    wxi = pwp.tile([128, M_TILES * 128], f16, tag="wxi")
                    wz = pwp.tile([128, M_TILES * 128], f16, tag="wz")
                    nc.scalar.dma_start(wxi[:], wxiT_d[:, m * EH:(m + 1) * EH])
                    nc.gpsimd.dma_start(wz[:], wzT_d[:, m * EH:(m + 1) * EH])
                    ps_xi = psA.tile([128, L], f32, tag="xi")
                    ps_z = psA.tile([128, L], f32, tag="z")
                    for kt in range(M_TILES):
                        xk = xT[:, kt * (3 + L):(kt + 1) * (3 + L)]
                        for h in range(2):
                            nc.tensor.matmul(ps_xi[:, h * 512:(h + 1) * 512],
                                             wxi[:, kt * 128:(kt + 1) * 128],
                                             xk[:, 3 + h * 512: 3 + (h + 1) * 512],
                                             start=(kt == 0), stop=(kt == M_TILES - 1))
                            nc.tensor.matmul(ps_z[:, h * 512:(h + 1) * 512],
                                             wz[:, kt * 128:(kt + 1) * 128],
                                             xk[:, 3 + h * 512: 3 + (h + 1) * 512],
                                             start=(kt == 0), stop=(kt == M_TILES - 1))
                    # conv in f16: 4 tensor_scalar taps + add tree (rotating slots)
                    xi16 = pxp.tile([128, 3 + L], f16, tag="xi16")
                    if m < 2:
                        nc.vector.memset(xi16[:, 0:3], 0.0)  # 2 slots, gap stays 0
                    nc.scalar.copy(xi16[:, 3:3 + L], ps_xi[:])
                    def tap(k):
                        t_ = pxp.tile([128, L], f16, tag="ct")
                        nc.vector.tensor_scalar_mul(t_[:], xi16[:, k:k + L],
                                                    convw[:, m * K + k:m * K + k + 1])
                        return t_
                    ta, tb = tap(0), tap(1)
                    s01 = pxp.tile([128, L], f16, tag="cs")
                    nc.vector.tensor_add(s01[:], ta[:], tb[:])
                    tc_, td = tap(2), tap(3)
                    s23 = pxp.tile([128, L], f16, tag="cs")
                    nc.vector.tensor_add(s23[:], tc_[:], td[:])
                    cacc = pxp.tile([128, L], f16, tag="cacc")
                    nc.vector.tensor_add(cacc[:], s01[:], s23[:])
                    nc.scalar.activation(xc16[:, m * L:(m + 1) * L], cacc[:],
                                         Act.Silu, bias=convb[:, m:m + 1])
                    nc.scalar.activation(sz16[:, m * L:(m + 1) * L], ps_z[:], Act.Silu)

            # ---------- Phase B: x_proj partial + single AllReduce + broadcasts ----------
            with tc.tile_pool(name="pb", bufs=1) as pbp, \
                 tc.tile_pool(name="pbw", bufs=2) as pbw, \
                 tc.tile_pool(name="psB", bufs=1, space="PSUM") as psB:
                ps_dbl = psB.tile([96, L], f32)
                for m in range(M_TILES):
                    xp = pbw.tile([128, 96], f16, tag="xp")
                    nc.sync.dma_start(xp[:], xpT_d[m * 128:(m + 1) * 128, :])
                    for h in range(2):
                        nc.tensor.matmul(ps_dbl[:, h * 512:(h + 1) * 512], xp[:],
                                         xc16[:, m * L + h * 512: m * L + (h + 1) * 512],
                                         start=(m == 0), stop=(m == M_TILES - 1))
                cvt16 = pbp.tile([96, L], f16)
                nc.scalar.copy(cvt16[:], ps_dbl[:])
                nc.sync.dma_start(dbl_in[:], cvt16[:])
                nc.gpsimd.collective_compute(
                    "AllReduce", Alu.add,
                    replica_groups=[[0, 1], [2, 3], [4, 5], [6, 7]],
                    ins=[dbl_in[:]], outs=[dbl_out[:]])
                nc.sync.dma_start(dblr16[:], dbl_out[:])
                bcrows = pbp.tile([32, L], f16)
                nc.scalar.dma_start(bcrows[:], dbl_out[64:96, :])
                # replicate B/C rows to 128 partitions: selector-matmul + evac
                with tc.tile_pool(name="psR", bufs=2, space="PSUM") as psR:
                    for n in range(N):
                        for sel_row, dst in ((n, bca), (16 + n, bcc)):
                            ps_r = psR.tile([128, L], f32, tag="rep")
                            for h in range(2):
                                nc.tensor.matmul(ps_r[:, h * 512:(h + 1) * 512],
                                                 sel32[:, sel_row * 128:(sel_row + 1) * 128],
                                                 bcrows[:, h * 512:(h + 1) * 512],
                                                 start=True, stop=True)
                            nc.scalar.copy(dst[:, n * L:(n + 1) * L], ps_r[:])

            # ---------- Phase C: delta (batched), dA planes, scan, y ----------
            with tc.tile_pool(name="pc", bufs=2) as pcp, \
                 tc.tile_pool(name="pc1", bufs=2) as pc1, \
                 tc.tile_pool(name="pcw", bufs=2) as pcw, \
                 tc.tile_pool(name="psC", bufs=1, space="PSUM") as psC, \
                 tc.tile_pool(name="psY", bufs=2, space="PSUM") as psY, \
                 tc.tile_pool(name="psD", bufs=1, space="PSUM") as psD:
                bca3 = bca[:].rearrange("p (n l) -> p n l", l=L)
                bcc3 = bcc[:].rearrange("p (n l) -> p n l", l=L)
                for m in range(M_TILES):
                    # delta = softplus(dt@dtw + dtb), just-in-time per m
                    ps_dt = psC.tile([128, L], f32, tag="dt")
                    for h in range(2):
                        nc.tensor.matmul(ps_dt[:, h * 512:(h + 1) * 512],
                                         dtwT[:, m * 128:(m + 1) * 128],
                                         dblr16[0:DTR, h * 512:(h + 1) * 512],
                                         start=True, stop=True)
                    ee16 = pcp.tile([128, L], f16, tag="ee16")
                    nc.scalar.activation(ee16[:], ps_dt[:], Act.Exp,
                                         bias=dtb[:, m:m + 1])
                    delta16 = pcp.tile([128, L], f16, tag="d16")
                    nc.scalar.activation(delta16[:], ee16[:], Act.Ln, bias=1.0)
                    u16 = pcp.tile([128, L], f16, tag="u16")
                    nc.vector.tensor_mul(u16[:], delta16[:],
                                         xc16[:, m * L:(m + 1) * L])
                    ps_y = psY.tile([128, L], f32, tag="ysum")
                    for nb in range(NB):
                        dA = pcp.tile([128, NPB * PL], f32, tag="dA")
                        dBu = pcp.tile([128, NPB * PL], f16, tag="dBu")
                        for j in range(NPB):
                            n = nb * NPB + j
                            nc.scalar.activation(dA[:, j * PL:j * PL + L],
                                                 delta16[:], Act.Exp,
                                                 scale=arate[:, m * N + n:m * N + n + 1])
                        dA3 = dA[:].rearrange("p (n l) -> p n l", l=PL)
                        dBu3 = dBu[:].rearrange("p (n l) -> p n l", l=PL)
                        if m == 0 and nb < 2:
                            # gap columns stay 0 across slot reuse (2 slots/tag)
                            nc.vector.memset(dA3[:, :, L:PL], 0.0)
                            nc.vector.memset(dBu3[:, :, L:PL], 0.0)
                        nc.vector.tensor_mul(
                            dBu3[:, :, 0:L],
                            u16[:, None, :].broadcast_to([128, NPB, L]),
                            bca3[:, nb * NPB:(nb + 1) * NPB, :])
                        h4 = pcp.tile([128, NPB * PL], f16, tag="h4")
                        nc.vector.tensor_tensor_scan(h4[:], dA[:], dBu[:], 0.0,
                                                     Alu.mult, Alu.add)
                        h43 = h4[:].rearrange("p (n l) -> p n l", l=PL)
                        prod = pcp.tile([128, NPB * PL], f16, tag="dBu")
                        prod3 = prod[:].rearrange("p (n l) -> p n l", l=PL)
                        nc.vector.tensor_mul(prod3[:, :, 0:L], h43[:, :, 0:L],
                                             bcc3[:, nb * NPB:(nb + 1) * NPB, :])
                        # sum the 16 n-planes on the tensor engine (identity
                        # matmul accumulating into PSUM)
                        for j in range(NPB):
                            for h in range(2):
                                nc.tensor.matmul(
                                    ps_y[:, h * 512:(h + 1) * 512], ident[:],
                                    prod[:, j * PL + h * 512: j * PL + (h + 1) * 512],
                                    start=(nb == 0 and j == 0),
                                    stop=(nb == NB - 1 and j == NPB - 1))
                    xdp = pc1.tile([128, L], f16, tag="xdp")
                    nc.vector.tensor_scalar_mul(
                        xdp[:], xc16[:, m * L:(m + 1) * L], dp[:, m:m + 1])
                    ys16 = pc1.tile([128, L], f16, tag="ys")
                    nc.vector.tensor_add(ys16[:], ps_y[:], xdp[:])
                    nc.vector.tensor_mul(g16[:, m * L:(m + 1) * L], ys16[:],
                                         sz16[:, m * L:(m + 1) * L])

                    # ---- out_proj pass-1 once g16[0:P1M] ready ----
                    if m == P1M - 1:
                        for dm in range(M_TILES):
                            wo = pcw.tile([128, M_TILES * 128], f16, tag="wo")
                            nc.sync.dma_start(wo[:], woT_d[:, dm * D:(dm + 1) * D])
                            ps_o = psD.tile([128, L], f32, tag="o")
                            for mm in range(P1M):
                                for h in range(2):
                                    nc.tensor.matmul(
                                        ps_o[:, h * 512:(h + 1) * 512],
                                        wo[:, mm * 128:(mm + 1) * 128],
                                        g16[:, mm * L + h * 512: mm * L + (h + 1) * 512],
                                        start=(mm == 0), stop=(mm == P1M - 1))
                            nc.scalar.copy(part16[:, dm * L:(dm + 1) * L], ps_o[:])

                # ---- out_proj pass-2 (m = P1M..7) + pair ReduceScatter ----
                for dm in range(M_TILES):
                    wo = pcw.tile([128, M_TILES * 128], f16, tag="wo")
                    nc.sync.dma_start(wo[:], woT_d[:, dm * D:(dm + 1) * D])
                    ps_o = psD.tile([128, L], f32, tag="o")
                    for mm in range(P1M, M_TILES):
                        for h in range(2):
                            nc.tensor.matmul(
                                ps_o[:, h * 512:(h + 1) * 512],
                                wo[:, mm * 128:(mm + 1) * 128],
                                g16[:, mm * L + h * 512: mm * L + (h + 1) * 512],
                                start=(mm == P1M), stop=(mm == M_TILES - 1))
                    oc16 = pcp.tile([128, L], f16, tag="oc16")
                    nc.vector.tensor_add(oc16[:], part16[:, dm * L:(dm + 1) * L], ps_o[:])
                    nc.sync.dma_start(out_d[dm * 128:(dm + 1) * 128, :], oc16[:])

    nc.compile()
    return nc


def _host_prep(inputs):
    """Build the 8 per-core input maps from the full problem inputs."""
    x = np.asarray(inputs["x"], np.float32)
    merge_w = np.asarray(inputs["merge_w"], np.float32)
    in_maps = []
    for b in range(B):
        for di, pre in enumerate(("fwd", "bwd")):
            p = {k: np.asarray(inputs[f"{pre}_{k}"], np.float32)
                 for k in ("in_proj", "conv_w", "conv_b", "x_proj", "dt_w",
                           "dt_b", "A_log", "D", "out_proj")}
            xb = x[b]
            if di == 1:
                xb = xb[::-1]
            xTp = np.concatenate([np.zeros((D, 3), np.float32), xb.T], axis=1)
            A = -np.exp(p["A_log"])                       # (E, N)
            W = merge_w[:, di * D:(di + 1) * D] @ p["out_proj"]   # (D, E)
            def pack_lhsT(wT):
                # (D, EH) -> [p, m*1024 + kt*128 + e']
                return np.ascontiguousarray(
                    wT.reshape(M_TILES, 128, M_TILES, 128).transpose(1, 2, 0, 3)
                    .reshape(128, M_TILES * EH))

            for half in range(2):
                sl = slice(half * EH, (half + 1) * EH)
                wxiT = pack_lhsT(p["in_proj"][:E][sl].T)
                wzT = pack_lhsT(p["in_proj"][E:][sl].T)
                convw = p["conv_w"][sl].reshape(M_TILES, 128, K).transpose(1, 0, 2).reshape(128, M_TILES * K)
                convb = p["conv_b"][sl].reshape(M_TILES, 128).T
                xpT = p["x_proj"][:, sl].T                # (EH, 96)
                dtwT = p["dt_w"][sl].T                    # (DTR, EH)
                dtb = p["dt_b"][sl].reshape(M_TILES, 128).T
                arate = A[sl].reshape(M_TILES, 128, N).transpose(1, 0, 2).reshape(128, M_TILES * N)
                dpv = p["D"][sl].reshape(M_TILES, 128).T
                woT = pack_lhsT(W[:, sl].T)               # (EH, D) pre-tiled
                in_maps.append({
                    "xT": xTp.astype(np.float16),
                    "wxiT": wxiT.astype(np.float16),
                    "wzT": wzT.astype(np.float16),
                    "convw": np.ascontiguousarray(convw, np.float32),
                    "convb": np.ascontiguousarray(convb, np.float32),
                    "xpT": xpT.astype(np.float16),
                    "dtwT": dtwT.astype(np.float16),
                    "dtb": np.ascontiguousarray(dtb, np.float32),
                    "arate": np.ascontiguousarray(arate, np.float32),
                    "dp": np.ascontiguousarray(dpv, np.float32),
                    "woT": woT.astype(np.float16),
                    "ident": np.eye(128, dtype=np.float16),
                    "sel32": np.repeat(np.eye(32, dtype=np.float16), 128,
                                       axis=1).reshape(32, 32 * 128),
                })
    return in_maps


def _ensure_neuron_platform():
    """If a caller pinned jax to cpu, re-point it at the neuron/axon PJRT
    platform so run_bass_kernel_spmd sees the 8 NeuronCores."""
    import jax
    try:
        if len(jax.devices()) >= 8 and jax.devices()[0].platform != "cpu":
            return
    except Exception:
        pass
    for plat in ("axon", "neuron"):
        try:
            jax.config.update("jax_platforms", plat)
            if len(jax.devices()) >= 8:
                return
        except Exception:
            continue


def kernel(**inputs):
    _ensure_neuron_platform()
    from concourse.bass_utils import run_bass_kernel_spmd
    if "nc" not in _nc_cache:
        _nc_cache["nc"] = _build_nc()
    nc = _nc_cache["nc"]
    in_maps = _host_prep(inputs)
    res = run_bass_kernel_spmd(nc, in_maps, core_ids=list(range(8)))
    _nc_cache["last_results"] = res
    # Stitch: each core returns its full [D, L] out_proj partial (own e-half,
    # merge_w folded); host sums halves + directions, un-flipping bwd time.
    out = np.zeros((B, L, D), np.float32)
    for b in range(B):
        acc = np.zeros((D, L), np.float32)
        for di in range(2):
            od = (res.results[4 * b + 2 * di + 0]["out_p"].astype(np.float32) +
                  res.results[4 * b + 2 * di + 1]["out_p"].astype(np.float32))
            if di == 1:
                od = od[:, ::-1]
            acc += od
        out[b] = acc.T
    return out
